# revision 10
# baseline (speedup 1.0000x reference)
"""Trainium2 Bass kernel for nn_ABSEncoder (dense_transformer).

Strategy: data-parallel over batch B=16 across 8 NeuronCores (2 batches/core).
Per batch (all sizes 1024 except yc=4096 tokens):
  E   = F_emb[x]                      # gather, [S=1024, D=1024]
  Y   = yc_r @ P_w + P_b              # [W=1024, 1024], contraction 4096
  A   = E @ Y + mask*(-30)            # logits [S, 1024]
  out = softmax(A) @ E                # [S, D]

All matmul operands in bf16 (f32 PSUM accumulation). Embedding gathers via
gpsimd dma_gather (f32 rows, one instruction per 512 tokens); all transposes
via SBUF-source dma_gather(transpose=True) on bf16 — zero TensorE transposes.
Host pre-marshals: int16-wrapped indices, bf16 P_w in [ki, ko, n] layout,
mask pre-scaled to -30*mask in bf16, iota index tables.
"""

import numpy as np
import ml_dtypes

BF16 = ml_dtypes.bfloat16

B = 16
NCORES = 8
BPC = B // NCORES          # batches per core
D = 1024                   # d_model == S == W
VOCAB = 32000
CTX = 4
YC = CTX * D               # 4096 yc tokens per batch
NEG = -30.0                # mask bias (exp(-30) ~ 1e-13, vs reference -1e9)


def _wrap16(t):
    """Wrap an int array [N] into dma_gather idx layout [128, N/16] int16:
    idx i lives at [i % 16, i // 16], replicated into all 8 16-partition
    groups (each GpSimd Q7 core reads its own group on HW)."""
    t = np.asarray(t)
    n = t.shape[-1]
    lead = t.shape[:-1]
    w = np.zeros(lead + (128, n // 16), dtype=np.int16)
    blk = np.swapaxes(t.reshape(lead + (n // 16, 16)), -1, -2)
    for k in range(8):
        w[..., 16 * k : 16 * k + 16, :] = blk
    return w


def _iota_perm512():
    # half-quarter transpose-gather permutation: column i2 = jj*128 + w_local
    # holds local token 4*w_local + jj
    i2 = np.arange(512)
    return _wrap16((4 * (i2 % 128) + (i2 // 128)).astype(np.int16))


def build_nc(bpc=BPC, stage=4):
    import concourse.tile as tile
    from concourse import bacc, mybir
    from contextlib import ExitStack

    f32 = mybir.dt.float32
    bf16 = mybir.dt.bfloat16
    i16 = mybir.dt.int16

    nc = bacc.Bacc("TRN2", target_bir_lowering=False, debug=False)

    # ---- DRAM parameters (per-core shard) ----
    F_emb = nc.dram_tensor("F_emb", [VOCAB, D], f32, kind="ExternalInput")
    G_emb = nc.dram_tensor("G_emb", [VOCAB, D], f32, kind="ExternalInput")
    Pw = nc.dram_tensor("Pw", [128, 32, D], bf16, kind="ExternalInput")
    Pb = nc.dram_tensor("Pb", [128, D], f32, kind="ExternalInput")
    x16 = nc.dram_tensor("x16", [bpc, 128, D // 16], i16, kind="ExternalInput")
    yc16 = nc.dram_tensor("yc16", [bpc, 128, YC // 16], i16, kind="ExternalInput")
    maskneg = nc.dram_tensor("maskneg", [bpc, D, D], bf16, kind="ExternalInput")
    iota_p = nc.dram_tensor("iota_p", [128, 32], i16, kind="ExternalInput")
    iota_s = nc.dram_tensor("iota_s", [128, 64], i16, kind="ExternalInput")
    iota_c = nc.dram_tensor("iota_c", [128, 8], i16, kind="ExternalInput")
    out = nc.dram_tensor("out", [bpc, D, D], f32, kind="ExternalOutput")

    add = mybir.AluOpType.add
    mult = mybir.AluOpType.mult
    Exp = mybir.ActivationFunctionType.Exp

    with tile.TileContext(nc) as tc, ExitStack() as ctx:
        pool = lambda name, bufs, **kw: ctx.enter_context(
            tc.tile_pool(name=name, bufs=bufs, **kw)
        )
        const_p = pool("const", 1)
        g32_p = pool("g32", 2)          # f32 gather staging [128,2,1024]
        gbf_p = pool("gbf", 2)          # bf16 cast staging  [128,4,1024]
        ycT_p = pool("ycT", 2)          # [128,8,512] per half-quarter
        e_p = pool("e", 1)              # E, ET, Y (big per-batch, bufs=1)
        eat_p = pool("eat", 3)          # expA^T chunk [128,8,128]
        expa_p = pool("expa", 2)        # expA chunk staging [128,1024]
        mask_p = pool("maskt", 2)
        am_p = pool("am", 2)
        o_p = pool("o", 2)
        st_p = pool("stats", 2)
        psum_p = pool("psum", 8, space="PSUM")

        # ---- persistent loads ----
        Pw_t = const_p.tile([128, 32, D], bf16)
        nc.sync.dma_start(Pw_t[:], Pw.ap())
        Pb_t = const_p.tile([128, D], f32)
        nc.sync.dma_start(Pb_t[:], Pb.ap())
        iota_p_t = const_p.tile([128, 32], i16)
        nc.sync.dma_start(iota_p_t[:], iota_p.ap())
        iota_s_t = const_p.tile([128, 64], i16)
        nc.sync.dma_start(iota_s_t[:], iota_s.ap())
        iota_c_t = const_p.tile([128, 8], i16)
        nc.sync.dma_start(iota_c_t[:], iota_c.ap())

        for b in range(bpc):
            yc16_t = const_p.tile([128, YC // 16], i16, tag="yc16t")
            nc.sync.dma_start(yc16_t[:], yc16.ap()[b])
            x16_t = const_p.tile([128, D // 16], i16, tag="x16t")
            nc.sync.dma_start(x16_t[:], x16.ap()[b])

            Y_sb = e_p.tile([128, 8, D], bf16, tag="Y")
            E_sb = e_p.tile([128, 8, D], bf16, tag="E")
            ET_h = [e_p.tile([128, 8, 512], bf16, tag=f"ET{hh}", name=f"ET{hh}")
                    for hh in range(2)]
            rsums = st_p.tile([128, 8, 2], f32, tag="rsums")
            rrec = st_p.tile([128, 8], f32, tag="rrec")

            # ===== yc pipeline + MM1: Y = yc_r @ P_w + P_b =====
            for h in range(8):          # half-quarters: 512 tokens, w-chunk h
                gbf = gbf_p.tile([128, 4, D], bf16, tag="gbf")
                for u in range(2):
                    g32 = g32_p.tile([128, 2, D], f32, tag="g32")
                    nc.gpsimd.dma_gather(
                        g32[:], G_emb.ap(),
                        yc16_t[:, 32 * h + 16 * u : 32 * h + 16 * u + 16],
                        256, 256, D)
                    nc.vector.tensor_copy(gbf[:, 2 * u : 2 * u + 2, :], g32[:])
                # transpose: ycT_h[p, qd, jj*128 + w_local] = emb(4w+jj)[qd*128+p]
                ycT_h = ycT_p.tile([128, 8, 512], bf16, tag="ycT")
                nc.gpsimd.dma_gather(
                    ycT_h[:], gbf[:],
                    iota_p_t[:], 512, 512, D,
                    transpose=True,
                    sbuf_tokens_per_rank=128,
                    sbuf_free_dim_per_rank=2 * D,
                )
                # MM1 for w-chunk m = h
                for jh in range(2 if stage >= 1 else 0):
                    ps = psum_p.tile([128, 512], f32, tag="ps")
                    for ko in range(32):
                        jj, qd = ko // 8, ko % 8
                        nc.tensor.matmul(
                            ps[:],
                            lhsT=ycT_h[:, qd, 128 * jj : 128 * jj + 128],
                            rhs=Pw_t[:, ko, 512 * jh : 512 * jh + 512],
                            start=(ko == 0), stop=(ko == 31),
                        )
                    nc.vector.tensor_tensor(
                        Y_sb[:, h, 512 * jh : 512 * jh + 512],
                        ps[:], Pb_t[:, 512 * jh : 512 * jh + 512], add)

            # ===== x pipeline: E, ET =====
            for h in range(4):
                g32 = g32_p.tile([128, 2, D], f32, tag="g32")
                nc.gpsimd.dma_gather(
                    g32[:], F_emb.ap(), x16_t[:, 16 * h : 16 * h + 16],
                    256, 256, D)
                nc.vector.tensor_copy(E_sb[:, 2 * h : 2 * h + 2, :], g32[:])
            for hh in range(2):
                # transposed x_e: ET_h[hh][p, qw, sl] = x_e[512*hh+sl, qw*128+p]
                nc.gpsimd.dma_gather(
                    ET_h[hh][:], E_sb[:], iota_s_t[:, 32 * hh : 32 * hh + 32],
                    512, 512, D,
                    transpose=True,
                    sbuf_tokens_per_rank=128,
                    sbuf_free_dim_per_rank=2 * D,
                )
            if stage < 4:
                # debug stages: dump staging so the output is written
                nc.sync.dma_start(
                    out.ap()[b, 0:256, :].rearrange("(g p) e -> p g e", p=128),
                    g32[:])

            # ===== per s-chunk m: MM2 + softmax + expA^T + MM3 =====
            for m in range(8 if stage >= 2 else 0):
                mt = mask_p.tile([128, D], bf16, tag="maskt")
                nc.sync.dma_start(mt[:], maskneg.ap()[b, 128 * m : 128 * m + 128, :])
                expa = expa_p.tile([128, D], bf16, tag="expa")
                for jh in range(2):
                    ps = psum_p.tile([128, 512], f32, tag="ps")
                    for k in range(8):
                        nc.tensor.matmul(
                            ps[:],
                            lhsT=ET_h[m // 4][:, k,
                                              128 * (m % 4) : 128 * (m % 4) + 128],
                            rhs=Y_sb[:, k, 512 * jh : 512 * jh + 512],
                            start=(k == 0), stop=(k == 7),
                        )
                    am = am_p.tile([128, 512], f32, tag="am")
                    nc.vector.tensor_tensor(
                        am[:], ps[:], mt[:, 512 * jh : 512 * jh + 512], add)
                    nc.scalar.activation(
                        expa[:, 512 * jh : 512 * jh + 512], am[:], Exp,
                        accum_out=rsums[:, m, jh : jh + 1])
                if stage < 3:
                    continue
                eAT_m = eat_p.tile([128, 8, 128], bf16, tag="eat")
                nc.gpsimd.dma_gather(
                    eAT_m[:], expa[:],
                    iota_c_t[:], 128, 128, D,
                    transpose=True,
                    sbuf_tokens_per_rank=128,
                    sbuf_free_dim_per_rank=2 * D,
                )
                # rrec[:, m] = 1 / (rsums[:,m,0] + rsums[:,m,1])
                nc.vector.tensor_tensor(
                    rrec[:, m : m + 1], rsums[:, m, 0:1], rsums[:, m, 1:2], add)
                nc.vector.reciprocal(rrec[:, m : m + 1], rrec[:, m : m + 1])

                # MM3 for this s-chunk: out[s, :] = (expA @ E) * rrec
                for dh in range(2 if stage >= 4 else 0):
                    ps = psum_p.tile([128, 512], f32, tag="ps")
                    for k in range(8):
                        nc.tensor.matmul(
                            ps[:],
                            lhsT=eAT_m[:, k, :],
                            rhs=E_sb[:, k, 512 * dh : 512 * dh + 512],
                            start=(k == 0), stop=(k == 7),
                        )
                    ot = o_p.tile([128, 512], f32, tag="ot")
                    nc.vector.tensor_scalar(
                        ot[:], ps[:], rrec[:, m : m + 1], None, op0=mult)
                    nc.sync.dma_start(
                        out.ap()[b, 128 * m : 128 * m + 128,
                                 512 * dh : 512 * dh + 512], ot[:])

        if stage < 4:
            for b in range(bpc):
                pass
    nc.compile()
    return nc


def host_prep(x, yc, mask, F_emb, G_emb, P_w, P_b, bpc=BPC, ncores=NCORES):
    """Marshal full inputs into per-core in_maps."""
    x = np.asarray(x)
    yc = np.asarray(yc)
    mask = np.asarray(mask)
    F_emb = np.ascontiguousarray(np.asarray(F_emb, dtype=np.float32))
    G_emb = np.ascontiguousarray(np.asarray(G_emb, dtype=np.float32))
    P_w = np.asarray(P_w, dtype=np.float32)
    P_b = np.asarray(P_b, dtype=np.float32)

    x16 = _wrap16(x.astype(np.int16))                    # [B, 128, 64]
    yc16 = _wrap16(yc.astype(np.int16))                  # [B, 128, 256]
    maskneg = (mask.astype(np.float32) * NEG).astype(BF16)   # [B, 1024, 1024]
    Pw_t = np.ascontiguousarray(
        P_w.astype(BF16).reshape(32, 128, D).transpose(1, 0, 2))  # [128,32,1024]
    Pb_t = np.ascontiguousarray(np.broadcast_to(P_b, (128, D)).astype(np.float32))
    iota_p = _iota_perm512()                             # [128, 32]
    iota_s = _wrap16(np.arange(1024, dtype=np.int16))    # [128, 64]
    iota_c = _wrap16(np.arange(128, dtype=np.int16))     # [128, 8]

    in_maps = []
    for c in range(ncores):
        sl = slice(c * bpc, (c + 1) * bpc)
        in_maps.append({
            "F_emb": F_emb,
            "G_emb": G_emb,
            "Pw": Pw_t,
            "Pb": Pb_t,
            "x16": np.ascontiguousarray(x16[sl]),
            "yc16": np.ascontiguousarray(yc16[sl]),
            "maskneg": np.ascontiguousarray(maskneg[sl]),
            "iota_p": iota_p,
            "iota_s": iota_s,
            "iota_c": iota_c,
        })
    return in_maps


_NC_CACHE = {}


def get_nc(bpc=BPC):
    if bpc not in _NC_CACHE:
        _NC_CACHE[bpc] = build_nc(bpc)
    return _NC_CACHE[bpc]


def kernel(x, yc, mask, training=0, F_emb=None, G_emb=None, P_w=None, P_b=None,
           _trace=False):
    from concourse.bass_utils import run_bass_kernel_spmd

    in_maps = host_prep(x, yc, mask, F_emb, G_emb, P_w, P_b)
    nc = get_nc()
    res = run_bass_kernel_spmd(nc, in_maps, core_ids=list(range(NCORES)),
                               trace=_trace)
    out = np.concatenate([r["out"] for r in res.results], axis=0)
    out = out.reshape(B, D, D)
    if _trace:
        kernel.last_result = res
    return out
